# revision 1
# baseline (speedup 1.0000x reference)
"""DCRNN (K=1, H0=0) fused kernel for 8 Trainium2 NeuronCores.

Math (derived from the reference with H0 = 0):
    R is dead (multiplied by H0=0); XH == XHR == [x, 0].
    Az = (Wz[0] + Wz[1])[:F]           # [256, 32]
    Ah = (Wh[0] + Wh[1])[:F]           # [256, 32]
    Zc = sigmoid(-(x @ Az + bz))       # == 1 - Z, strictly positive
    T  = tanh(x @ Ah + bh)
    h  = relu(Zc * T) == Zc * relu(T)
    y  = h @ Wl + bl                   # [N, 1]

Strategy: data-parallel over nodes on 8 cores.  Per core, x-shard rows are
DMA-transpose-loaded (bf16) so features sit on partitions; each 128-node
subtile of x.T is the *stationary* matmul operand against the small moving
weight [128, 64] = [Az|Ah] chunk, so pre-activations land in natural
orientation [128 nodes, 64] in PSUM.  Biases are added with a K=1 rank-1
matmul (ones ⊗ biascat).  ScalarE applies sigmoid(-pre)/tanh straight out
of PSUM; VectorE fuses relu+mult, applies Wl and row-reduces to y.  y is
PE-transposed at the end so the store is one contiguous DMA.
"""

import sys

import numpy as np

sys.path.insert(0, "/opt/trn_rl_repo")

import ml_dtypes

N = 200000
F = 256
HID = 32
NCORES = 8
PER = 25088            # padded nodes per core: 25088 = 24*1024 + 512
NPAD = PER * NCORES    # 200704
SUPER = 1024           # nodes per superblock (8 subtiles of 128)
NSUPER = 25            # 24 full superblocks + 1 half (512 nodes)
YCOLS = PER // 128     # 196

BF16 = ml_dtypes.bfloat16

_PROGS = {}
VARIANT = "hostT2"  # best measured; hostT4 (pair-coalesced) was ~2us slower


def _build_program(reps=1):
    import contextlib

    import concourse.tile as tile
    from concourse import bacc, mybir

    BF = mybir.dt.bfloat16
    F32 = mybir.dt.float32
    AF = mybir.ActivationFunctionType
    OP = mybir.AluOpType

    nc = bacc.Bacc("TRN2", target_bir_lowering=False, debug=False,
                   num_devices=NCORES)

    if VARIANT in ("hostT2", "hostT4"):
        # host feeds per-superblock transposed contiguous blocks
        x_d = nc.dram_tensor("x", [2 * PER * 128], BF, kind="ExternalInput").ap()
    elif VARIANT == "hostT":
        # host feeds x already transposed: row f = feature, col = node
        x_d = nc.dram_tensor("x", [F, PER], BF, kind="ExternalInput").ap()
    elif VARIANT == "hostperm":
        # host pre-permutes x so each (superblock, chunk) transpose source
        # is one contiguous [nn, 128] block
        x_d = nc.dram_tensor("x", [2 * PER, 128], BF, kind="ExternalInput").ap()
    else:
        x_d = nc.dram_tensor("x", [PER, F], BF, kind="ExternalInput").ap()
    acat_d = nc.dram_tensor("acat", [2, 128, 64], BF, kind="ExternalInput").ap()
    bias_d = nc.dram_tensor("biascat", [1, 512], BF, kind="ExternalInput").ap()
    wl_d = nc.dram_tensor("wlfull", [128, 256], BF, kind="ExternalInput").ap()
    ones_d = nc.dram_tensor("ones", [1, 128], BF, kind="ExternalInput").ap()
    id_d = nc.dram_tensor("ident", [128, 128], F32, kind="ExternalInput").ap()
    y_d = nc.dram_tensor("y", [YCOLS, 128], F32, kind="ExternalOutput").ap()

    with tile.TileContext(nc) as tc:
        with tc.tile_pool(name="const", bufs=1) as cp, \
             tc.tile_pool(name="xt", bufs=8) as xp, \
             tc.tile_pool(name="act", bufs=6) as vp, \
             tc.tile_pool(name="ps", bufs=6, space="PSUM") as pp, \
             tc.tile_pool(name="yps", bufs=2, space="PSUM") as yp:

            acat0 = cp.tile([128, 64], BF)
            acat1 = cp.tile([128, 64], BF)
            biascat = cp.tile([1, 512], BF)
            wlfull = cp.tile([128, 256], BF)
            ones = cp.tile([1, 128], BF)
            ident = cp.tile([128, 128], F32)
            ysb = cp.tile([128, YCOLS], F32)

            nc.scalar.dma_start(out=acat0[:], in_=acat_d[0])
            nc.scalar.dma_start(out=acat1[:], in_=acat_d[1])
            nc.scalar.dma_start(out=biascat[:], in_=bias_d[:])
            nc.scalar.dma_start(out=wlfull[:], in_=wl_d[:])
            nc.scalar.dma_start(out=ones[:], in_=ones_d[:])
            nc.scalar.dma_start(out=ident[:], in_=id_d[:])

            rep_ctx = (tc.For_i(0, reps, 1,
                               hint_engines=(mybir.EngineType.PE,
                                             mybir.EngineType.SP))
                       if reps > 1 else contextlib.nullcontext())
            with rep_ctx:
                _kernel_body(nc, tc, mybir, BF, F32, AF, OP,
                             x_d, y_d, xp, vp, pp, yp,
                             acat0, acat1, biascat, wlfull, ones, ident, ysb)

    nc.compile()
    return nc


def _kernel_body(nc, tc, mybir, BF, F32, AF, OP, x_d, y_d, xp, vp, pp, yp,
                 acat0, acat1, biascat, wlfull, ones, ident, ysb):
    _pair = {}
    if True:
        if True:
            for b in range(NSUPER):
                nsub = 8 if b < NSUPER - 1 else 4
                nn = nsub * 128
                base = b * SUPER

                if VARIANT == "hostT4":
                    if b < NSUPER - 1:
                        g, k = divmod(b, 2)
                        if k == 0:
                            xt4 = xp.tile([128, 4096], BF, tag="xt")
                            goff = g * 524288
                            nc.sync.dma_start(
                                out=xt4[:, 0:2048].rearrange(
                                    "p (k j) -> p k j", k=2),
                                in_=x_d[goff:goff + 262144].rearrange(
                                    "(k p j) -> p k j", k=2, p=128))
                            nc.gpsimd.dma_start(
                                out=xt4[:, 2048:4096].rearrange(
                                    "p (k j) -> p k j", k=2),
                                in_=x_d[goff + 262144:goff + 524288].rearrange(
                                    "(k p j) -> p k j", k=2, p=128))
                            _pair[0] = xt4
                        xt4 = _pair[0]

                        def _lhs(s, c, xt4=xt4, k=k):
                            o = c * 2048 + k * 1024 + s * 128
                            return xt4[:, o:o + 128]
                    else:
                        xt = xp.tile([128, 4096], BF, tag="xt")
                        off = 12 * 524288
                        nc.sync.dma_start(
                            out=xt[:, :2 * nn].rearrange(
                                "p (c j) -> p c j", c=2),
                            in_=x_d[off:off + 256 * nn].rearrange(
                                "(c p j) -> p c j", c=2, p=128))

                        def _lhs(s, c, xt=xt, nn=nn):
                            return xt[:, c * nn + s * 128:c * nn + (s + 1) * 128]
                elif VARIANT == "hostT2":
                    xt = xp.tile([128, 2 * SUPER], BF, tag="xt")
                    off = base * 256
                    nc.sync.dma_start(
                        out=xt[:, :nn],
                        in_=x_d[off:off + 128 * nn].rearrange(
                            "(p j) -> p j", p=128))
                    nc.gpsimd.dma_start(
                        out=xt[:, nn:2 * nn],
                        in_=x_d[off + 128 * nn:off + 256 * nn].rearrange(
                            "(p j) -> p j", p=128))

                    def _lhs(s, c, xt=xt, nn=nn):
                        return xt[:, c * nn + s * 128:c * nn + (s + 1) * 128]
                elif VARIANT == "hostT":
                    xt0t = xp.tile([128, SUPER], BF, tag="xt0")
                    xt1t = xp.tile([128, SUPER], BF, tag="xt1")
                    nc.sync.dma_start(out=xt0t[:, :nn],
                                      in_=x_d[0:128, base:base + nn])
                    nc.sync.dma_start(out=xt1t[:, :nn],
                                      in_=x_d[128:256, base:base + nn])

                    def _lhs(s, c, xt0=xt0t, xt1=xt1t):
                        t = xt0 if c == 0 else xt1
                        return t[:, s * 128:(s + 1) * 128]
                elif VARIANT == "hostperm":
                    xt0t = xp.tile([128, SUPER], BF, tag="xt0")
                    xt1t = xp.tile([128, SUPER], BF, tag="xt1")
                    r0 = 2 * base
                    nc.sync.dma_start(out=xt0t[:, :nn],
                                      in_=x_d[r0:r0 + nn, :],
                                      transpose=True)
                    nc.sync.dma_start(out=xt1t[:, :nn],
                                      in_=x_d[r0 + nn:r0 + 2 * nn, :],
                                      transpose=True)

                    def _lhs(s, c, xt0=xt0t, xt1=xt1t):
                        t = xt0 if c == 0 else xt1
                        return t[:, s * 128:(s + 1) * 128]
                elif VARIANT in ("inter2", "nocompute"):
                    # two contiguous-source transposes; columns interleave
                    # (node, chunk) pairs
                    x2 = x_d.rearrange("n (a c) -> (n a) c", c=128)
                    tA = xp.tile([128, SUPER], BF, tag="xtA")
                    tB = xp.tile([128, SUPER], BF, tag="xtB")
                    nc.sync.dma_start(out=tA[:, :nn],
                                      in_=x2[2 * base:2 * base + nn, :],
                                      transpose=True)
                    nc.sync.dma_start(out=tB[:, :nn],
                                      in_=x2[2 * base + nn:2 * base + 2 * nn, :],
                                      transpose=True)
                    tA3 = tA[:, :nn].rearrange("p (j c) -> p c j", c=2)
                    tB3 = tB[:, :nn].rearrange("p (j c) -> p c j", c=2)
                    half_sub = nsub // 2

                    def _lhs(s, c, tA3=tA3, tB3=tB3, half_sub=half_sub):
                        t3 = tA3 if s < half_sub else tB3
                        j0 = (s % half_sub) * 128
                        return t3[:, c, j0:j0 + 128]
                elif VARIANT == "inter":
                    # single contiguous-source transpose; even columns are
                    # feature chunk 0, odd columns chunk 1
                    x2 = x_d.rearrange("n (a c) -> (n a) c", c=128)
                    xti = xp.tile([128, 2 * SUPER], BF, tag="xti")
                    nc.sync.dma_start(out=xti[:, :2 * nn],
                                      in_=x2[2 * base:2 * (base + nn), :],
                                      transpose=True)
                    xt3 = xti[:, :2 * nn].rearrange("p (j c) -> p c j", c=2)

                    def _lhs(s, c, xt3=xt3):
                        return xt3[:, c, s * 128:(s + 1) * 128]
                elif VARIANT == "plainload":
                    # timing probe only: same bytes, no transpose (wrong data)
                    xt0 = xp.tile([128, SUPER], BF, tag="xt0")
                    xt1 = xp.tile([128, SUPER], BF, tag="xt1")
                    xv = x_d[base:base + nn, :].rearrange(
                        "(p a) f -> p (a f)", p=128)
                    nc.sync.dma_start(out=xt0[:, :nn], in_=xv[:, :nn])
                    nc.sync.dma_start(out=xt1[:, :nn], in_=xv[:, nn:2 * nn])

                    def _lhs(s, c, xt0=xt0, xt1=xt1):
                        t = xt0 if c == 0 else xt1
                        return t[:, s * 128:(s + 1) * 128]
                else:
                    xt0t = xp.tile([128, SUPER], BF, tag="xt0")
                    xt1t = xp.tile([128, SUPER], BF, tag="xt1")
                    nc.sync.dma_start(out=xt0t[:, :nn],
                                      in_=x_d[base:base + nn, 0:128],
                                      transpose=True)
                    nc.sync.dma_start(out=xt1t[:, :nn],
                                      in_=x_d[base:base + nn, 128:256],
                                      transpose=True)

                    def _lhs(s, c, xt0=xt0t, xt1=xt1t):
                        t = xt0 if c == 0 else xt1
                        return t[:, s * 128:(s + 1) * 128]

                if VARIANT == "nocompute":
                    # timing probe: force DMA completion with tiny reads
                    nc.vector.tensor_copy(ysb[:, b:b + 1], _lhs(0, 0)[:, 0:1])
                    nc.vector.tensor_copy(ysb[:, b:b + 1], _lhs(0, 1)[:, 0:1])
                    continue

                ps = pp.tile([128, 512], F32, tag="ps")
                # rank-1 bias broadcast fills the bank and opens the group
                nc.tensor.matmul(ps[:, :nsub * 64], ones[:],
                                 biascat[:, :nsub * 64],
                                 start=True, stop=False)
                for s in range(nsub):
                    out_sl = ps[:, s * 64:(s + 1) * 64]
                    nc.tensor.matmul(out_sl, _lhs(s, 0), acat0[:],
                                     start=False, stop=False)
                    nc.tensor.matmul(out_sl, _lhs(s, 1), acat1[:],
                                     start=False, stop=(s == nsub - 1))

                ps3 = ps[:, :nsub * 64].rearrange("p (s h) -> p s h", h=64)
                zc = vp.tile([128, 256], BF, tag="zc")
                tt = vp.tile([128, 256], BF, tag="tt")
                zc3 = zc[:, :nsub * 32].rearrange("p (s h) -> p s h", h=32)
                tt3 = tt[:, :nsub * 32].rearrange("p (s h) -> p s h", h=32)
                nc.scalar.activation(zc3, ps3[:, :, 0:32], AF.Sigmoid,
                                     scale=-1.0)
                nc.scalar.activation(tt3, ps3[:, :, 32:64], AF.Tanh)

                gr = vp.tile([128, 256], BF, tag="gr")
                gw = vp.tile([128, 256], BF, tag="gw")
                # gr = relu(tt) * zc  (zc > 0 so this equals relu(zc*tt))
                nc.vector.scalar_tensor_tensor(
                    gr[:, :nsub * 32], tt[:, :nsub * 32], 0.0,
                    zc[:, :nsub * 32], op0=OP.max, op1=OP.mult)
                nc.vector.tensor_mul(gw[:, :nsub * 32], gr[:, :nsub * 32],
                                     wlfull[:, :nsub * 32])
                gw3 = gw[:, :nsub * 32].rearrange("p (s h) -> p s h", h=32)
                nc.vector.tensor_reduce(ysb[:, b * 8:b * 8 + nsub], gw3,
                                        axis=mybir.AxisListType.X, op=OP.add)

                # flush finished halves of ysb mid-loop to shorten the tail
                if b == 11 or b == NSUPER - 1:
                    h0 = 0 if b == 11 else 96
                    hw = 96 if b == 11 else YCOLS - 96  # 96 then 100
                    ytp = yp.tile([128, 128], F32, tag="ytp")
                    nc.tensor.transpose(ytp[:hw, :],
                                        ysb[:, h0:h0 + hw], ident[:])
                    yts = vp.tile([128, 128], F32, tag="yts")
                    nc.vector.tensor_copy(yts[:hw, :], ytp[:hw, :])
                    nc.sync.dma_start(out=y_d[h0:h0 + hw, :],
                                      in_=yts[:hw, :])


def _get_program(reps=1):
    if reps not in _PROGS:
        _PROGS[reps] = _build_program(reps)
    return _PROGS[reps]


def _host_inputs(x, Wz, bz, Wh, bh, Wl):
    Az = (np.asarray(Wz[0]) + np.asarray(Wz[1]))[:F]
    Ah = (np.asarray(Wh[0]) + np.asarray(Wh[1]))[:F]
    Acat = np.concatenate([Az, Ah], axis=1)               # [256, 64]
    acat = np.stack([Acat[:128], Acat[128:]]).astype(BF16)
    biascat = np.concatenate([np.asarray(bz), np.asarray(bh)])  # [64]
    biascat8 = np.tile(biascat, 8)[None, :].astype(BF16)  # [1, 512]
    wlfull = np.tile(np.asarray(Wl).reshape(1, HID), (128, 8)).astype(BF16)
    ones = np.ones((1, 128), BF16)
    ident = np.eye(128, dtype=np.float32)

    xb = np.zeros((NPAD, F), dtype=BF16)
    xb[:N] = np.asarray(x).astype(BF16)
    shards = xb.reshape(NCORES, PER, F)
    if VARIANT == "hostT4":
        nfull = (NSUPER - 1) * SUPER  # 24576
        main = shards[:, :nfull].reshape(NCORES, 12, 2, SUPER, 2, 128)
        # (g, k, j, c, f) -> (g, c, k, f, j)
        main = main.transpose(0, 1, 4, 2, 5, 3).reshape(NCORES, -1)
        tail = shards[:, nfull:].reshape(NCORES, 1, PER - nfull, F)
        tail = tail.transpose(0, 1, 3, 2).reshape(NCORES, -1)
        shards = np.concatenate([main, tail], axis=1)  # [NCORES, 2*PER*128]
    elif VARIANT == "hostT2":
        nfull = (NSUPER - 1) * SUPER
        main = shards[:, :nfull].reshape(NCORES, NSUPER - 1, SUPER, F)
        main = main.transpose(0, 1, 3, 2).reshape(NCORES, -1)
        tail = shards[:, nfull:].reshape(NCORES, 1, PER - nfull, F)
        tail = tail.transpose(0, 1, 3, 2).reshape(NCORES, -1)
        shards = np.concatenate([main, tail], axis=1)  # [NCORES, 2*PER*128]
    elif VARIANT == "hostT":
        # [NCORES, PER, F] -> [NCORES, F, PER]
        shards = np.ascontiguousarray(shards.transpose(0, 2, 1))
    elif VARIANT == "hostperm":
        # [(b sup) (c f)] -> [(b c sup) f]: every (superblock, chunk)
        # transpose source becomes one contiguous [sup, 128] block
        nfull = (NSUPER - 1) * SUPER  # 24576
        main = shards[:, :nfull].reshape(NCORES, NSUPER - 1, SUPER, 2, 128)
        main = main.transpose(0, 1, 3, 2, 4).reshape(NCORES, -1, 128)
        tail = shards[:, nfull:].reshape(NCORES, 1, PER - nfull, 2, 128)
        tail = tail.transpose(0, 1, 3, 2, 4).reshape(NCORES, -1, 128)
        shards = np.concatenate([main, tail], axis=1)  # [NCORES, 2*PER, 128]
    return shards, acat, biascat8, wlfull, ones, ident


def kernel(x, edge_index, Wz, bz, Wr, br, Wh, bh, Wl, bl, _reps=1):
    from concourse.bass_utils import run_bass_kernel_spmd

    shards, acat, biascat8, wlfull, ones, ident = _host_inputs(
        x, Wz, bz, Wh, bh, Wl)

    nc = _get_program(_reps)
    in_maps = [{
        "x": np.ascontiguousarray(shards[i]),
        "acat": acat,
        "biascat": biascat8,
        "wlfull": wlfull,
        "ones": ones,
        "ident": ident,
    } for i in range(NCORES)]

    res = run_bass_kernel_spmd(nc, in_maps, core_ids=list(range(NCORES)))

    y = np.concatenate([np.asarray(res.results[i]["y"]).reshape(-1)
                        for i in range(NCORES)])[:N]
    out = (y + np.float32(np.asarray(bl).reshape(-1)[0])).astype(np.float32)
    return out.reshape(N, 1)



# revision 2
# speedup vs baseline: 1.2070x; 1.2070x over previous
"""DCRNN (K=1, H0=0) fused kernel for 8 Trainium2 NeuronCores — fp8 v2.

Math (derived from the reference with H0 = 0):
    R is dead (multiplied by H0=0); XH == XHR == [x, 0].
    Az = (Wz[0] + Wz[1])[:F]           # [256, 32]
    Ah = (Wh[0] + Wh[1])[:F]           # [256, 32]
    Zc = sigmoid(-(x @ Az + bz))       # == 1 - Z, strictly positive
    T  = tanh(x @ Ah + bh)             # == 2*sigmoid(2*(x @ Ah + bh)) - 1
    h  = relu(Zc * T)
    y  = h @ Wl + bl                   # [N, 1]

v2 strategy (per core, data-parallel over nodes):
  * x is quantized host-side to fp8 e3m4 (xq = e3m4(c*x), c ~ 15.5/max|x|),
    transposed host-side to [128 feat-of-chunk, (superblock, chunk, node)]
    so DMA rows are contiguous.  HBM traffic halves vs bf16.
  * The whole 6.4 MB shard is preloaded to SBUF via a few ~1MB DMAs.
  * Matmuls are node-stationary: lhsT = fp8 x-chunk [128, 128] (FWL 4x
    weight load), rhs = folded bf16 weights acat' = [-Az/c | 2*Ah/c]
    (scale and sign baked in), accumulate over the 2 feature chunks.
  * tanh(u) = 2*sigmoid(2u) - 1 lets ONE Sigmoid ACT instruction per
    1024-col psum group produce both gates: Zc = sg_z, T' = sg_t.
  * DVE: u = (T' - 0.5)*Zc;  g = max(u,0) * (2*Wl)  (Zc > 0 makes the
    relu placement exact);  tree-add reduce over HID -> ysb columns.
  * y is PE-transposed in two flushes and stored with one DMA each.
  * bz/bh are zero in this model; a generic variant adds them with a
    rank-1 matmul when any bias is nonzero.
"""

import sys

import numpy as np

sys.path.insert(0, "/opt/trn_rl_repo")

import ml_dtypes

N = 200000
F = 256
HID = 32
NCORES = 8
PER = 25088            # padded nodes per core
NPAD = PER * NCORES    # 200704
SUPER = 1024           # nodes per superblock (8 subtiles of 128)
NSUPER = 25            # 24 full superblocks + 1 half (512 nodes)
YCOLS = PER // 128     # 196

# x DMA groups, in superblocks (ramped: small first so compute starts early)
XGROUPS = [1, 3, 4, 4, 4, 4, 4, 1]
# ACT groups: pairs of superblocks (last group is the lone half-super)
ACTG = [(2 * i, 2 * i + 1) for i in range(12)] + [(24,)]

BF16 = ml_dtypes.bfloat16
E3M4 = ml_dtypes.float8_e3m4
E3MAX = 15.5

_PROGS = {}


def _build_program(with_bias=False):
    import concourse.tile as tile
    from concourse import bacc, mybir

    BF = mybir.dt.bfloat16
    F8 = mybir.dt.float8e3
    F32 = mybir.dt.float32
    AF = mybir.ActivationFunctionType
    OP = mybir.AluOpType

    nc = bacc.Bacc("TRN2", target_bir_lowering=False, debug=False,
                   num_devices=NCORES)

    # host layout: [128, 50176] fp8; col = (b*2048 + c*1024 + j) for b<24,
    # tail b=24: 49152 + c*512 + j
    x_d = nc.dram_tensor("x", [128, 2 * PER], F8, kind="ExternalInput").ap()
    acat_d = nc.dram_tensor("acat", [2, 128, 64], BF, kind="ExternalInput").ap()
    wl2_d = nc.dram_tensor("wl2full", [128, 1024], BF, kind="ExternalInput").ap()
    id_d = nc.dram_tensor("ident", [128, 128], F32, kind="ExternalInput").ap()
    bias_d = nc.dram_tensor("biascat", [1, 512], BF, kind="ExternalInput").ap()
    ones_d = nc.dram_tensor("ones", [1, 128], BF, kind="ExternalInput").ap()
    y_d = nc.dram_tensor("y", [YCOLS, 128], F32, kind="ExternalOutput").ap()

    with tile.TileContext(nc) as tc:
        with tc.tile_pool(name="const", bufs=1) as cp, \
             tc.tile_pool(name="xs", bufs=len(XGROUPS)) as xp, \
             tc.tile_pool(name="sg", bufs=2) as gp, \
             tc.tile_pool(name="dv", bufs=2) as vp, \
             tc.tile_pool(name="ps", bufs=2, space="PSUM") as pp, \
             tc.tile_pool(name="yps", bufs=2, space="PSUM") as yp:

            acat0 = cp.tile([128, 64], BF)
            acat1 = cp.tile([128, 64], BF)
            wl2full = cp.tile([128, 1024], BF)
            ident = cp.tile([128, 128], F32)
            ysb = cp.tile([128, YCOLS], F32)
            nc.scalar.dma_start(out=acat0[:], in_=acat_d[0])
            nc.scalar.dma_start(out=acat1[:], in_=acat_d[1])
            nc.scalar.dma_start(out=wl2full[:], in_=wl2_d[:])
            nc.scalar.dma_start(out=ident[:], in_=id_d[:])
            if with_bias:
                biascat = cp.tile([1, 512], BF)
                ones = cp.tile([1, 128], BF)
                nc.scalar.dma_start(out=biascat[:], in_=bias_d[:])
                nc.scalar.dma_start(out=ones[:], in_=ones_d[:])

            # ---- preload the whole x shard (ramped DMA sizes, one queue)
            xtiles = []      # (tile, first_super, n_supers)
            b0 = 0
            for ng in XGROUPS:
                xt = xp.tile([128, 8192], F8, tag="xt")
                c0 = b0 * 2048
                w = sum(2048 if (b0 + i) < NSUPER - 1 else 1024
                        for i in range(ng))
                nc.sync.dma_start(out=xt[:, :w], in_=x_d[:, c0:c0 + w])
                xtiles.append((xt, b0, ng))
                b0 += ng

            def lhs(b, c, s):
                """stationary fp8 x chunk [128, 128] for (superblock b,
                feature-chunk c, subtile s)."""
                for xt, g0, ng in xtiles:
                    if g0 <= b < g0 + ng:
                        nn = 1024 if b < NSUPER - 1 else 512
                        off = (b - g0) * 2048 + c * nn + s * 128
                        return xt[:, off:off + 128]
                raise AssertionError(b)

            # ---- main loop over ACT groups (2 superblocks each)
            ydone = 0
            for gi, supers in enumerate(ACTG):
                ncols = sum((1024 if b < NSUPER - 1 else 512) // 2
                            for b in supers)   # 64 cols per 128-node subtile
                pt = pp.tile([128, 1024], F32, tag="pt")
                col = 0
                for b in supers:
                    nsub = 8 if b < NSUPER - 1 else 4
                    for s in range(nsub):
                        out_sl = pt[:, col:col + 64]
                        if with_bias:
                            nc.tensor.matmul(out_sl, ones[:],
                                             biascat[:, :64],
                                             start=True, stop=False)
                        nc.tensor.matmul(out_sl, lhs(b, 0, s), acat0[:],
                                         start=not with_bias, stop=False)
                        nc.tensor.matmul(out_sl, lhs(b, 1, s), acat1[:],
                                         start=False, stop=True)
                        col += 64

                # one sigmoid for both gates (z: sg=Zc, t: sg=T'=(T+1)/2)
                sg = gp.tile([128, 1024], BF, tag="sg")
                nc.scalar.activation(sg[:, :ncols], pt[:, :ncols], AF.Sigmoid)

                # u = (T' - 0.5) * Zc ; g = max(u, 0) * (2*Wl)
                nsubg = ncols // 64
                sg3 = sg[:, :ncols].rearrange("p (s h) -> p s h", h=64)
                u = vp.tile([128, 512], BF, tag="u")
                u3 = u[:, :32 * nsubg].rearrange("p (s h) -> p s h", h=32)
                nc.vector.scalar_tensor_tensor(u3, sg3[:, :, 32:64], 0.5,
                                               sg3[:, :, 0:32],
                                               op0=OP.subtract, op1=OP.mult)
                g = vp.tile([128, 512], BF, tag="g")
                nc.vector.scalar_tensor_tensor(
                    g[:, :32 * nsubg], u[:, :32 * nsubg], 0.0,
                    wl2full[:, :32 * nsubg], op0=OP.max, op1=OP.mult)

                # tree reduce over HID=32 -> one col per subtile
                g3 = g[:, :32 * nsubg].rearrange("p (s h) -> p s h", h=32)
                t1 = vp.tile([128, 256], BF, tag="t1")
                t13 = t1[:, :16 * nsubg].rearrange("p (s h) -> p s h", h=16)
                nc.vector.tensor_add(t13, g3[:, :, 0:16], g3[:, :, 16:32])
                t2 = vp.tile([128, 128], BF, tag="t2")
                t23 = t2[:, :8 * nsubg].rearrange("p (s h) -> p s h", h=8)
                nc.vector.tensor_add(t23, t13[:, :, 0:8], t13[:, :, 8:16])
                t3 = vp.tile([128, 64], BF, tag="t3")
                t33 = t3[:, :4 * nsubg].rearrange("p (s h) -> p s h", h=4)
                nc.vector.tensor_add(t33, t23[:, :, 0:4], t23[:, :, 4:8])
                t4 = vp.tile([128, 32], BF, tag="t4")
                t43 = t4[:, :2 * nsubg].rearrange("p (s h) -> p s h", h=2)
                nc.vector.tensor_add(t43, t33[:, :, 0:2], t33[:, :, 2:4])
                yc0 = supers[0] * 8
                y3 = ysb[:, yc0:yc0 + nsubg].rearrange("p (s h) -> p s h", h=1)
                nc.vector.tensor_add(y3, t43[:, :, 0:1], t43[:, :, 1:2])
                ydone = yc0 + nsubg

                # flush finished halves of ysb mid-loop to shorten the tail
                if ydone == 96 or gi == len(ACTG) - 1:
                    h0 = 0 if ydone == 96 else 96
                    hw = 96 if ydone == 96 else YCOLS - 96
                    ytp = yp.tile([128, 128], F32, tag="ytp")
                    nc.tensor.transpose(ytp[:hw, :], ysb[:, h0:h0 + hw],
                                        ident[:])
                    yts = vp.tile([128, 128], F32, tag="yts")
                    nc.vector.tensor_copy(yts[:hw, :], ytp[:hw, :])
                    nc.scalar.dma_start(out=y_d[h0:h0 + hw, :],
                                        in_=yts[:hw, :])

    nc.compile()
    return nc


def _get_program(with_bias=False):
    if with_bias not in _PROGS:
        _PROGS[with_bias] = _build_program(with_bias)
    return _PROGS[with_bias]


def _host_inputs(x, Wz, bz, Wh, bh, Wl):
    x = np.asarray(x)
    Az = (np.asarray(Wz[0]) + np.asarray(Wz[1]))[:F]
    Ah = (np.asarray(Wh[0]) + np.asarray(Wh[1]))[:F]

    c = E3MAX / max(float(np.abs(x).max()), 1e-30)
    c = min(c, 1e30)
    Acat = np.concatenate([-Az / c, (2.0 / c) * Ah], axis=1)  # [256, 64]
    acat = np.stack([Acat[:128], Acat[128:]]).astype(BF16)    # [2, 128, 64]
    wl2full = np.tile(np.asarray(Wl).reshape(1, HID) * 2.0,
                      (128, 32)).astype(BF16)                 # [128, 1024]
    ident = np.eye(128, dtype=np.float32)
    biascat = np.concatenate([-np.asarray(bz), 2.0 * np.asarray(bh)])
    biascat8 = np.tile(biascat, 8)[None, :].astype(BF16)      # [1, 512]
    ones = np.ones((1, 128), BF16)

    # quantize + per-core transpose to [128, (b, c, j)] layout
    xq = np.clip(x * c, -E3MAX, E3MAX).astype(E3M4)
    xb = np.zeros((NPAD, F), dtype=E3M4)
    xb[:N] = xq
    shards = xb.reshape(NCORES, PER, F)
    nfull = (NSUPER - 1) * SUPER                              # 24576
    # main: [PER0, 256] -> [24, 1024, 2, 128] -> [128, 24, 2, 1024]
    main = shards[:, :nfull].reshape(NCORES, NSUPER - 1, SUPER, 2, 128)
    main = main.transpose(0, 4, 1, 3, 2).reshape(NCORES, 128, -1)
    tail = shards[:, nfull:].reshape(NCORES, 1, PER - nfull, 2, 128)
    tail = tail.transpose(0, 4, 1, 3, 2).reshape(NCORES, 128, -1)
    xhost = np.concatenate([main, tail], axis=2)              # [NC, 128, 2*PER]
    return xhost, acat, wl2full, ident, biascat8, ones


def kernel(x, edge_index, Wz, bz, Wr, br, Wh, bh, Wl, bl):
    from concourse.bass_utils import run_bass_kernel_spmd

    xhost, acat, wl2full, ident, biascat8, ones = _host_inputs(
        x, Wz, bz, Wh, bh, Wl)
    with_bias = bool(np.any(np.asarray(bz)) or np.any(np.asarray(bh)))

    nc = _get_program(with_bias)
    in_maps = [{
        "x": np.ascontiguousarray(xhost[i]),
        "acat": acat,
        "wl2full": wl2full,
        "ident": ident,
        "biascat": biascat8,
        "ones": ones,
    } for i in range(NCORES)]

    res = run_bass_kernel_spmd(nc, in_maps, core_ids=list(range(NCORES)))

    y = np.concatenate([np.asarray(res.results[i]["y"]).reshape(-1)
                        for i in range(NCORES)])[:N]
    out = (y + np.float32(np.asarray(bl).reshape(-1)[0])).astype(np.float32)
    return out.reshape(N, 1)


# revision 3
# speedup vs baseline: 1.3358x; 1.1067x over previous
"""DCRNN (K=1, H0=0) fused kernel for 8 Trainium2 NeuronCores — fp8 v3.

Math (derived from the reference with H0 = 0):
    R is dead (multiplied by H0=0); XH == XHR == [x, 0].
    Az = (Wz[0] + Wz[1])[:F]           # [256, 32]
    Ah = (Wh[0] + Wh[1])[:F]           # [256, 32]
    Zc = sigmoid(-(x @ Az + bz))       # == 1 - Z, strictly positive
    T  = tanh(x @ Ah + bh)
    y  = (relu(Zc * T) @ Wl) + bl      # relu(Zc*T) == Zc>0 ? relu-placement exact

v3 strategy (per core, data-parallel over nodes):
  * x quantized host-side to fp8 e3m4 (xq = e3m4(c*x)); sign/scale folded
    into bf16 weights acat' = [-Az/c | Ah/c].  HBM traffic halves vs bf16.
  * Whole 6.4MB shard preloaded to SBUF with 6 large DMAs on one HWDGE queue.
  * Node-stationary matmuls: lhsT = fp8 x-chunk [128,128] (FWL 4x weight
    load), rhs = bf16 acat chunk [128,64], psum groups of 3 superblocks.
  * ScalarE: Sigmoid on z-cols, Tanh on t-cols (one table set), strided
    psum reads -> CONTIGUOUS sgz/sgt tiles so DVE runs in 2x/4x modes.
  * VectorE: u = sgt*sgz (2x), r = relu(u) (4x), g = r*Wl (2x), tree adds
    t1/t2 (2x).  GpSimdE (otherwise idle) finishes the tree: t3/t4/y.
  * y is PE-transposed in three flushes and stored via the scalar queue.
  * bz/bh are zero here; a generic variant adds them with a rank-1 matmul.
"""

import sys

import numpy as np

sys.path.insert(0, "/opt/trn_rl_repo")

import ml_dtypes

N = 200000
F = 256
HID = 32
NCORES = 8
PER = 25088            # padded nodes per core
NPAD = PER * NCORES    # 200704
SUPER = 1024           # nodes per superblock (8 subtiles of 128)
NSUPER = 25            # 24 full superblocks + 1 half (512 nodes)
YCOLS = PER // 128     # 196

XGROUPS = [2, 4, 6, 6, 6, 1]                    # x DMA groups (superblocks)
ACTG = [tuple(range(3 * i, 3 * i + 3)) for i in range(8)] + [(24,)]

BF16 = ml_dtypes.bfloat16
E3M4 = ml_dtypes.float8_e3m4
E3MAX = 15.5

_PROGS = {}


def _build_program(with_bias=False):
    import concourse.tile as tile
    from concourse import bacc, mybir

    BF = mybir.dt.bfloat16
    F8 = mybir.dt.float8e3
    F32 = mybir.dt.float32
    AF = mybir.ActivationFunctionType
    OP = mybir.AluOpType

    nc = bacc.Bacc("TRN2", target_bir_lowering=False, debug=False,
                   num_devices=NCORES)

    # host layout: [128, 50176] fp8; col = (b*2048 + c*1024 + j) for b<24,
    # tail b=24: 49152 + c*512 + j
    x_d = nc.dram_tensor("x", [128, 2 * PER], F8, kind="ExternalInput").ap()
    acat_d = nc.dram_tensor("acat", [2, 128, 64], BF, kind="ExternalInput").ap()
    wl_d = nc.dram_tensor("wlfull", [128, 768], BF, kind="ExternalInput").ap()
    id_d = nc.dram_tensor("ident", [128, 128], F32, kind="ExternalInput").ap()
    bias_d = nc.dram_tensor("biascat", [1, 512], BF, kind="ExternalInput").ap()
    ones_d = nc.dram_tensor("ones", [1, 128], BF, kind="ExternalInput").ap()
    y_d = nc.dram_tensor("y", [YCOLS, 128], F32, kind="ExternalOutput").ap()

    with tile.TileContext(nc) as tc:
        with tc.tile_pool(name="const", bufs=1) as cp, \
             tc.tile_pool(name="xs", bufs=len(XGROUPS)) as xp, \
             tc.tile_pool(name="sg", bufs=4) as gp, \
             tc.tile_pool(name="dv", bufs=10) as vp, \
             tc.tile_pool(name="ps", bufs=2, space="PSUM") as pp, \
             tc.tile_pool(name="yps", bufs=2, space="PSUM") as yp:

            acat0 = cp.tile([128, 64], BF)
            acat1 = cp.tile([128, 64], BF)
            wlfull = cp.tile([128, 768], BF)
            ident = cp.tile([128, 128], F32)
            ysb = cp.tile([128, YCOLS], F32)
            nc.scalar.dma_start(out=acat0[:], in_=acat_d[0])
            nc.scalar.dma_start(out=acat1[:], in_=acat_d[1])
            nc.scalar.dma_start(out=wlfull[:], in_=wl_d[:])
            nc.scalar.dma_start(out=ident[:], in_=id_d[:])
            if with_bias:
                biascat = cp.tile([1, 512], BF)
                ones = cp.tile([1, 128], BF)
                nc.scalar.dma_start(out=biascat[:], in_=bias_d[:])
                nc.scalar.dma_start(out=ones[:], in_=ones_d[:])

            # ---- preload the whole x shard (large DMAs, one HWDGE queue)
            xtiles = []      # (tile, first_super, n_supers)
            b0 = 0
            for ng in XGROUPS:
                xt = xp.tile([128, 12288], F8, tag="xt")
                c0 = b0 * 2048
                w = sum(2048 if (b0 + i) < NSUPER - 1 else 1024
                        for i in range(ng))
                nc.sync.dma_start(out=xt[:, :w], in_=x_d[:, c0:c0 + w])
                xtiles.append((xt, b0, ng))
                b0 += ng

            def lhs(b, c, s):
                """stationary fp8 x chunk [128, 128] for (superblock b,
                feature-chunk c, subtile s)."""
                for xt, g0, ng in xtiles:
                    if g0 <= b < g0 + ng:
                        nn = 1024 if b < NSUPER - 1 else 512
                        off = (b - g0) * 2048 + c * nn + s * 128
                        return xt[:, off:off + 128]
                raise AssertionError(b)

            # ---- main loop over ACT groups (3 superblocks each)
            ydone = 0
            yflush = 0
            for gi, supers in enumerate(ACTG):
                nsubg = sum((8 if b < NSUPER - 1 else 4) for b in supers)
                ncols = nsubg * 64
                pt = pp.tile([128, 1536], F32, tag="pt")
                col = 0
                for b in supers:
                    nsub = 8 if b < NSUPER - 1 else 4
                    for s in range(nsub):
                        out_sl = pt[:, col:col + 64]
                        if with_bias:
                            nc.tensor.matmul(out_sl, ones[:],
                                             biascat[:, :64],
                                             start=True, stop=False)
                        nc.tensor.matmul(out_sl, lhs(b, 0, s), acat0[:],
                                         start=not with_bias, stop=False)
                        nc.tensor.matmul(out_sl, lhs(b, 1, s), acat1[:],
                                         start=False, stop=True)
                        col += 64

                # split gates: sigmoid(z-cols) / tanh(t-cols), strided psum
                # reads, contiguous SBUF writes
                nz = nsubg * 32
                pt3 = pt[:, :ncols].rearrange("p (s h) -> p s h", h=64)
                sgz = gp.tile([128, 768], BF, tag="sgz")
                sgt = gp.tile([128, 768], BF, tag="sgt")
                sgz3 = sgz[:, :nz].rearrange("p (s h) -> p s h", h=32)
                sgt3 = sgt[:, :nz].rearrange("p (s h) -> p s h", h=32)
                nc.scalar.activation(sgz3, pt3[:, :, 0:32], AF.Sigmoid)
                nc.scalar.activation(sgt3, pt3[:, :, 32:64], AF.Tanh)

                # DVE: u = sgt*sgz (2x), r = relu(u) (4x), g = r*Wl (2x)
                u = vp.tile([128, 768], BF, tag="u")
                nc.vector.tensor_mul(u[:, :nz], sgt[:, :nz], sgz[:, :nz])
                r = vp.tile([128, 768], BF, tag="r")
                nc.vector.tensor_scalar_max(r[:, :nz], u[:, :nz], 0.0)
                g = vp.tile([128, 768], BF, tag="g")
                nc.vector.tensor_mul(g[:, :nz], r[:, :nz], wlfull[:, :nz])

                # tree reduce over HID=32: t1,t2 on DVE; t3,t4,y on GpSimd
                g3 = g[:, :nz].rearrange("p (s h) -> p s h", h=32)
                t1 = vp.tile([128, 384], BF, tag="t1")
                t13 = t1[:, :16 * nsubg].rearrange("p (s h) -> p s h", h=16)
                nc.vector.tensor_add(t13, g3[:, :, 0:16], g3[:, :, 16:32])
                t2 = vp.tile([128, 192], BF, tag="t2")
                t23 = t2[:, :8 * nsubg].rearrange("p (s h) -> p s h", h=8)
                nc.vector.tensor_add(t23, t13[:, :, 0:8], t13[:, :, 8:16])
                t3 = vp.tile([128, 96], BF, tag="t3")
                t33 = t3[:, :4 * nsubg].rearrange("p (s h) -> p s h", h=4)
                nc.gpsimd.tensor_add(t33, t23[:, :, 0:4], t23[:, :, 4:8])
                t4 = vp.tile([128, 48], BF, tag="t4")
                t43 = t4[:, :2 * nsubg].rearrange("p (s h) -> p s h", h=2)
                nc.gpsimd.tensor_add(t43, t33[:, :, 0:2], t33[:, :, 2:4])
                yc0 = supers[0] * 8
                y3 = ysb[:, yc0:yc0 + nsubg].rearrange("p (s h) -> p s h", h=1)
                nc.gpsimd.tensor_add(y3, t43[:, :, 0:1], t43[:, :, 1:2])
                ydone = yc0 + nsubg

                # flush finished slices of ysb to shorten the tail
                while yflush < len(_FLUSH) and ydone >= _FLUSH[yflush][1]:
                    h0, h1 = _FLUSH[yflush]
                    hw = h1 - h0
                    ytp = yp.tile([128, 128], F32, tag="ytp")
                    nc.tensor.transpose(ytp[:hw, :], ysb[:, h0:h1], ident[:])
                    yts = vp.tile([128, 128], F32, tag="yts")
                    nc.vector.tensor_copy(yts[:hw, :], ytp[:hw, :])
                    nc.scalar.dma_start(out=y_d[h0:h1, :], in_=yts[:hw, :])
                    yflush += 1

    nc.compile()
    return nc


_FLUSH = [(0, 96), (96, 192), (192, 196)]


def _get_program(with_bias=False):
    if with_bias not in _PROGS:
        _PROGS[with_bias] = _build_program(with_bias)
    return _PROGS[with_bias]


def _host_inputs(x, Wz, bz, Wh, bh, Wl):
    x = np.asarray(x)
    Az = (np.asarray(Wz[0]) + np.asarray(Wz[1]))[:F]
    Ah = (np.asarray(Wh[0]) + np.asarray(Wh[1]))[:F]

    c = E3MAX / max(float(np.abs(x).max()), 1e-30)
    Acat = np.concatenate([-Az / c, Ah / c], axis=1)          # [256, 64]
    acat = np.stack([Acat[:128], Acat[128:]]).astype(BF16)    # [2, 128, 64]
    wlfull = np.tile(np.asarray(Wl).reshape(1, HID),
                     (128, 24)).astype(BF16)                  # [128, 768]
    ident = np.eye(128, dtype=np.float32)
    biascat = np.concatenate([-np.asarray(bz), np.asarray(bh)])
    biascat8 = np.tile(biascat, 8)[None, :].astype(BF16)      # [1, 512]
    ones = np.ones((1, 128), BF16)

    # quantize + per-core transpose to [128, (b, c, j)] layout
    xq = np.clip(x * c, -E3MAX, E3MAX).astype(E3M4)
    xb = np.zeros((NPAD, F), dtype=E3M4)
    xb[:N] = xq
    shards = xb.reshape(NCORES, PER, F)
    nfull = (NSUPER - 1) * SUPER                              # 24576
    main = shards[:, :nfull].reshape(NCORES, NSUPER - 1, SUPER, 2, 128)
    main = main.transpose(0, 4, 1, 3, 2).reshape(NCORES, 128, -1)
    tail = shards[:, nfull:].reshape(NCORES, 1, PER - nfull, 2, 128)
    tail = tail.transpose(0, 4, 1, 3, 2).reshape(NCORES, 128, -1)
    xhost = np.concatenate([main, tail], axis=2)              # [NC, 128, 2*PER]
    return xhost, acat, wlfull, ident, biascat8, ones


def kernel(x, edge_index, Wz, bz, Wr, br, Wh, bh, Wl, bl):
    from concourse.bass_utils import run_bass_kernel_spmd

    xhost, acat, wlfull, ident, biascat8, ones = _host_inputs(
        x, Wz, bz, Wh, bh, Wl)
    with_bias = bool(np.any(np.asarray(bz)) or np.any(np.asarray(bh)))

    nc = _get_program(with_bias)
    in_maps = [{
        "x": np.ascontiguousarray(xhost[i]),
        "acat": acat,
        "wlfull": wlfull,
        "ident": ident,
        "biascat": biascat8,
        "ones": ones,
    } for i in range(NCORES)]

    res = run_bass_kernel_spmd(nc, in_maps, core_ids=list(range(NCORES)))

    y = np.concatenate([np.asarray(res.results[i]["y"]).reshape(-1)
                        for i in range(NCORES)])[:N]
    out = (y + np.float32(np.asarray(bl).reshape(-1)[0])).astype(np.float32)
    return out.reshape(N, 1)
